# revision 37
# baseline (speedup 1.0000x reference)
"""Trainium2 Bass kernel for nn_CombSub (DDSP-style comb-subtractive vocoder).

Strategy: data-parallel over batch B=8 across 8 NeuronCores (1 row/core).
Per core, the whole pipeline runs on-device in fp32:
  - XLA-CPU-bit-exact 16-ary tree cumsum for the phase accumulator
  - sinc comb-tooth via exact range reduction + ACT Sin
  - transformer control net in [feat, frames] layout (features on partitions)
  - all FFT filtering as composite DFT matmuls with the overlap-add folded
    into three shifted irfft matrices (no scatter-add needed)
Host only marshals layouts (transposes inputs/outputs, precomputes
input-independent DFT matrices).
"""
import os
import numpy as np

import bass_rust
import concourse.bass as bass
import concourse.mybir as mybir
from concourse.tile import TileContext
from concourse.bass_utils import run_bass_kernel_spmd
from concourse.masks import make_identity

F32 = mybir.dt.float32
AF = mybir.ActivationFunctionType
OP = mybir.AluOpType

SR = 44100
BLOCK = 512
F = 800
T = F * BLOCK
D = 256
HEADS = 8
HD = 32
FFD = 1024
NOUT = 1024
MAGIC = 12582912.0        # 1.5 * 2**23 : (x+M)-M == round-half-even(x), |x|<2^22
NCHUNK = 400              # frame chunk for PSUM free dim
CHUNKS = (0, 400)
FT = [128, 128, 128, 128, 128, 128, 32]   # frame tiles (sum 800)
FT0 = [0, 128, 256, 384, 512, 640, 768]


# --------------------------------------------------------------------------
# walrus in this container encodes at most ONE sync-wait per instruction;
# peel extra waits onto same-engine NoOps (engine programs run in order).
def legalize_waits(nc, max_waits=1):
    n = 0
    for f in nc.m.functions:
        for blk in f.blocks:
            out, changed = [], False
            for ins in blk.instructions:
                si = ins.sync_info
                if si is not None and len(si.on_wait) > max_waits:
                    waits = list(si.on_wait)
                    for j, w in enumerate(waits[:-max_waits]):
                        nop = mybir.InstNoOp(name=f"wsplit-{ins.name}-{j}", ins=[], outs=[])
                        nop.engine = ins.engine
                        nop.sync_info = bass_rust.SyncInfo(on_update=[], on_wait=[w])
                        out.append(nop)
                    ins.sync_info = bass_rust.SyncInfo(
                        on_update=list(si.on_update), on_wait=waits[-max_waits:])
                    changed, n = True, n + 1
                out.append(ins)
            if changed:
                blk.instructions = out
    return n


# --------------------------------------------------------------------------
# constants (input-independent, f64 -> f32)
_CONSTS = None


def build_consts():
    global _CONSTS
    if _CONSTS is not None:
        return _CONSTS
    C = {}
    j = np.arange(BLOCK, dtype=np.float64)
    wj = (j / BLOCK)
    C['ups_A'] = (1.0 - wj).astype(np.float32)[None, :]      # (1, 512)
    C['ups_B'] = wj.astype(np.float32)[None, :]

    def irfft_mats(nbins, n):
        eye = np.eye(nbins)
        return (np.fft.irfft(eye, n=n, axis=1),
                np.fft.irfft(1j * eye, n=n, axis=1))

    def rfft_mats(nin, n):
        Fc = np.fft.rfft(np.eye(nin), n=n, axis=1)
        return Fc.real, Fc.imag

    G_re, G_im = irfft_mats(256, 510)
    roll = np.roll(np.eye(510), 255, axis=1)
    Fr, Fi = rfft_mats(510, 1022)
    C['M1_rr'] = (G_re @ roll @ Fr).astype(np.float32)       # (256, 512)
    C['M1_ri'] = (G_re @ roll @ Fi).astype(np.float32)
    C['M1_ir'] = (G_im @ roll @ Fr).astype(np.float32)
    C['M1_ii'] = (G_im @ roll @ Fi).astype(np.float32)

    n = np.arange(510)
    hann510 = 0.5 - 0.5 * np.cos(2.0 * np.pi * n / 510)
    C['N3_re'] = ((G_re @ roll) * hann510[None, :] @ Fr / 128.0).astype(np.float32)
    C['N3_im'] = ((G_re @ roll) * hann510[None, :] @ Fi / 128.0).astype(np.float32)

    G2_re, _ = irfft_mats(512, 1022)
    roll2 = np.roll(np.eye(1022), 511, axis=1)
    P2 = (G2_re @ roll2)                                     # (512, 1022)
    C['P2'] = np.zeros((512, 1024), np.float32)
    C['P2'][:, :1022] = P2.astype(np.float32)                # pad K to 1024
    F2r, F2i = rfft_mats(1022, 1534)                         # (1022, 768)
    C['F2_re'] = np.zeros((1024, 768), np.float32)
    C['F2_re'][:1022] = F2r.astype(np.float32)
    C['F2_im'] = np.zeros((1024, 768), np.float32)
    C['F2_im'][:1022] = F2i.astype(np.float32)

    FR1r, FR1i = rfft_mats(512, 1022)                        # (512, 512)
    C['FR1_re'] = FR1r.astype(np.float32)
    C['FR1_im'] = FR1i.astype(np.float32)
    FR2r, FR2i = rfft_mats(512, 1534)                        # (512, 768)
    C['FR2_re'] = FR2r.astype(np.float32)
    C['FR2_im'] = FR2i.astype(np.float32)

    def ola_mats(nbins, nfft, delay):
        IYre, IYim = irfft_mats(nbins, nfft)
        def sel(IY, shift):
            M = np.zeros((nbins, 512))
            for s in range(512):
                o = s + delay + shift
                if 0 <= o < nfft:
                    M[:, s] = IY[:, o]
            return M
        return {'A_re': sel(IYre, 0), 'A_im': sel(IYim, 0),
                'B_re': sel(IYre, 512), 'B_im': sel(IYim, 512),
                'C_re': sel(IYre, -512), 'C_im': sel(IYim, -512)}

    for k, v in ola_mats(512, 1022, 255).items():
        C[f'O1_{k}'] = v.astype(np.float32)                  # (512, 512) each
    for k, v in ola_mats(768, 1534, 511).items():
        C[f'O2_{k}'] = v.astype(np.float32)                  # (768, 512) each

    C['tcol'] = np.zeros((1024,), np.float32)
    C['tcol'][:1022] = (np.arange(1022) - 511).astype(np.float32)
    C['ones_row'] = np.ones((1, 128), np.float32)            # K=1 bcast lhsT
    C['ones_col'] = np.ones((128, 1), np.float32)            # partition-sum lhsT
    # head indicator: E4[h, p] = 1 if p//32 == h  (4, 128)
    E4 = np.zeros((4, 128), np.float32)
    for h in range(4):
        E4[h, h * 32:(h + 1) * 32] = 1.0
    C['E4'] = E4
    # groupnorm one-hots per partition tile: (2, 128, 2) fwd, (2, 2, 128) bcast
    G = np.zeros((2, 128, 2), np.float32)
    G[:, :64, 0] = 1.0
    G[:, 64:, 1] = 1.0
    C['gn_fwd'] = G.reshape(256, 2).copy()                   # (256, 2) lhsT per tile
    Gb = np.zeros((2, 128), np.float32)
    Gb[0, :64] = 1.0
    Gb[1, 64:] = 1.0
    C['gn_bc'] = Gb                                          # (2, 128): local grp -> partition
    # pi-scaled cumsum blocks for allpass phase: L[m', m] = pi if m' <= m
    L = np.triu(np.full((256, 256), np.pi)).astype(np.float32)
    C['piL'] = L                                             # (256, 256) lhsT
    _CONSTS = C
    return C


def prep_weights(params):
    """Host-side weight marshalling (layout only)."""
    f32 = lambda a: np.ascontiguousarray(np.asarray(a, dtype=np.float32))
    W = {}
    W['conv1_k'] = np.ascontiguousarray(
        np.transpose(f32(params['conv1_w']), (2, 1, 0)))     # (3, I, O)
    W['conv1_b'] = f32(params['conv1_b'])
    W['gn_g'] = f32(params['gn_g'])
    W['gn_b'] = f32(params['gn_b'])
    W['conv2_k'] = np.ascontiguousarray(
        np.transpose(f32(params['conv2_w']), (2, 1, 0)))
    W['bias2'] = (f32(params['conv2_b']) + f32(params['f0_b'])
                  + f32(params['phase_b']) + f32(params['vol_b']))
    W['f0_w'] = f32(params['f0_w'])                          # (1, 256)
    W['phase_w'] = f32(params['phase_w'])
    W['vol_w'] = f32(params['vol_w'])
    s = np.float32(1.0 / np.sqrt(HD))
    for li, lp in enumerate(params['layers']):
        wqkv = f32(lp['wqkv']).copy()
        bqkv = f32(lp['bqkv']).copy()
        wqkv[:, :256] *= s                                   # fold score scale into Q
        bqkv[:256] *= s
        W[f'l{li}_ln1_g'] = f32(lp['ln1_g'])
        W[f'l{li}_ln1_b'] = f32(lp['ln1_b'])
        W[f'l{li}_wqkv'] = wqkv
        W[f'l{li}_bqkv'] = bqkv
        W[f'l{li}_wo'] = f32(lp['wo'])
        W[f'l{li}_bo'] = f32(lp['bo'])
        W[f'l{li}_ln2_g'] = f32(lp['ln2_g'])
        W[f'l{li}_ln2_b'] = f32(lp['ln2_b'])
        W[f'l{li}_w1'] = f32(lp['w1'])
        W[f'l{li}_b1'] = f32(lp['b1'])
        W[f'l{li}_w2'] = f32(lp['w2'])
        W[f'l{li}_b2'] = f32(lp['b2'])
    W['ln_g'] = f32(params['ln_g'])
    W['ln_b'] = f32(params['ln_b'])
    W['out_w'] = f32(params['out_w'])                        # (256, 1024)
    W['out_b'] = f32(params['out_b'])
    return W


# --------------------------------------------------------------------------
def rowbc(ap, p=128):
    """DMA source AP broadcasting a DRAM row across p partitions."""
    return bass.AP(tensor=ap.tensor, offset=ap.offset, ap=[[0, p]] + list(ap.ap))


def build_program(taps=()):
    """Build the per-core Bass program. `taps` = iterable of intermediate
    names to expose as extra outputs for stage testing."""
    C = build_consts()
    taps = set(taps)
    nc = bass.Bass()
    DT = {}

    def din(name, shape):
        DT[name] = nc.dram_tensor(name, list(shape), F32, kind="ExternalInput")
        return DT[name]

    def dscratch(name, shape):
        DT[name] = nc.dram_tensor(name, list(shape), F32)
        return DT[name]

    def dout(name, shape):
        DT[name] = nc.dram_tensor(name, list(shape), F32, kind="ExternalOutput")
        return DT[name]

    # ---- inputs -----------------------------------------------------------
    din('units_T', (256, 800))
    din('f0_ext', (801,))            # f0 row with edge hold
    din('step_in', (T,))             # (upsampled f0)/SR in reference op order
    din('vol', (800,))
    din('noise_T', (512, 800))       # (noise*2-1 handled on device)
    # constants
    for k in ('ups_A', 'ups_B', 'M1_rr', 'M1_ri', 'M1_ir', 'M1_ii',
              'N3_re', 'N3_im', 'P2', 'F2_re', 'F2_im', 'FR1_re', 'FR1_im',
              'FR2_re', 'FR2_im', 'tcol', 'ones_row', 'ones_col', 'E4',
              'gn_fwd', 'gn_bc', 'piL'):
        din(k, C[k].shape)
    for t in ('O1', 'O2'):
        for k in ('A_re', 'A_im', 'B_re', 'B_im', 'C_re', 'C_im'):
            din(f'{t}_{k}', C[f'{t}_{k}'].shape)
    # weights
    din('conv1_k', (3, 256, 256)); din('conv1_b', (256,))
    din('gn_g', (256,)); din('gn_b', (256,))
    din('conv2_k', (3, 256, 256)); din('bias2', (256,))
    din('f0_w', (1, 256)); din('phase_w', (1, 256)); din('vol_w', (1, 256))
    for li in range(3):
        din(f'l{li}_ln1_g', (256,)); din(f'l{li}_ln1_b', (256,))
        din(f'l{li}_wqkv', (256, 768)); din(f'l{li}_bqkv', (768,))
        din(f'l{li}_wo', (256, 256)); din(f'l{li}_bo', (256,))
        din(f'l{li}_ln2_g', (256,)); din(f'l{li}_ln2_b', (256,))
        din(f'l{li}_w1', (256, 1024)); din(f'l{li}_b1', (1024,))
        din(f'l{li}_w2', (1024, 256)); din(f'l{li}_b2', (256,))
    din('ln_g', (256,)); din('ln_b', (256,))
    din('out_w', (256, 1024)); din('out_b', (1024,))

    # ---- scratch ----------------------------------------------------------
    dscratch('den_t', (T,))
    dscratch('rotw_t', (T,))
    dscratch('ct_t', (T,))
    dscratch('c1buf', (25600,))
    dscratch('l3buf', (112,))
    dscratch('o2buf', (104,))
    dscratch('o1buf', (1604,))
    dscratch('o0buf', (25604,))
    dscratch('phrow', (800,))
    dscratch('ct_sf', (512, 800))
    dscratch('h1_d', (512, 800))
    dscratch('harm_d', (512, 800))
    dscratch('tgd_d', (256, 800))
    dscratch('src_d', (512, 800))
    dscratch('npar_d', (256, 800))

    # ---- outputs ----------------------------------------------------------
    dout('o_signal', (512, 800))
    dout('o_phase', (800,))
    dout('o_harm', (512, 800))
    dout('o_noise', (512, 800))
    tap_outs = {}
    def tap(name, shape):
        if name in taps:
            tap_outs[name] = dout(f'tap_{name}', shape)
            return True
        return False

    with TileContext(nc) as tc:
        _build_stages(nc, tc, DT, tap, taps)
    legalize_waits(nc)
    return nc


def _build_stages(nc, tc, DT, tap, taps):
    from contextlib import ExitStack

    def ts(out, in0, s1, s2=None, op0=OP.add, op1=None, **kw):
        if op1 is None:
            return nc.vector.tensor_scalar(out, in0, s1, None, op0, **kw)
        return nc.vector.tensor_scalar(out, in0, s1, s2, op0, op1, **kw)

    def tt(out, a, b, op):
        return nc.vector.tensor_tensor(out, a, b, op)

    def act(out, in_, f, bias=0.0, scale=1.0, accum_out=None):
        return nc.scalar.activation(out, in_, f, bias=bias, scale=scale,
                                    accum_out=accum_out)

    def magic_round(pool, dst, src, tag):
        tmp = pool.tile([src.shape[0], src.shape[1]], F32, tag="mrtmp", name="mrtmp")
        ts(tmp, src, MAGIC, None, OP.add)
        ts(dst, tmp, MAGIC, None, OP.subtract)

    # ======================================================================
    # S1a: frame-layout f0 upsample -> step_t / den_t  (DRAM, t-order)
    # ======================================================================
    with ExitStack() as ctx:
        pool = ctx.enter_context(tc.tile_pool(name="s1a", bufs=2))
        upsA = pool.tile([128, 512], F32)
        upsB = pool.tile([128, 512], F32)
        nc.sync.dma_start(upsA, rowbc(DT['ups_A'][0]))
        nc.sync.dma_start(upsB, rowbc(DT['ups_B'][0]))
        for it in range(7):
            rows, r0 = FT[it], FT0[it]
            for nm_src, nm_dst, add_c in (('f0_ext', 'den_t', 0.001),):
                c0 = pool.tile([128, 1], F32, tag="c0", name="c0")
                c1 = pool.tile([128, 1], F32, tag="c1", name="c1")
                nc.sync.dma_start(c0[:rows], DT[nm_src][r0:r0 + rows, None])
                nc.sync.dma_start(c1[:rows], DT[nm_src][r0 + 1:r0 + 1 + rows, None])
                t1 = pool.tile([128, 512], F32, tag="t1", name="t1")
                t2 = pool.tile([128, 512], F32, tag="t2", name="t2")
                ts(t1[:rows], upsA[:rows], c0[:rows, 0:1], None, OP.mult)
                ts(t2[:rows], upsB[:rows], c1[:rows, 0:1], None, OP.mult)
                tt(t1[:rows], t1[:rows], t2[:rows], OP.add)
                if add_c is not None:
                    ts(t1[:rows], t1[:rows], add_c, None, OP.add)
                nc.sync.dma_start(
                    DT[nm_dst].rearrange("(f s) -> f s", s=512)[r0:r0 + rows], t1[:rows])

    # ======================================================================
    # S1b: XLA-CPU-replica cumsum of step over T; wrap; combtooth (t-layout)
    # ======================================================================
    with ExitStack() as ctx:
        pool = ctx.enter_context(tc.tile_pool(name="s1b", bufs=1))
        s0 = pool.tile([128, 3200], F32)
        nc.sync.dma_start(s0, DT['step_in'].rearrange("(p n) -> p n", n=3200))
        x0 = pool.tile([128, 3200], F32)
        nc.vector.tensor_copy(x0, s0)
        if tap('step', (128, 3200)):
            nc.sync.dma_start(DT['tap_step'][:], x0)
        v0 = s0.rearrange("p (g s) -> p g s", s=16)
        w0 = x0.rearrange("p (g s) -> p g s", s=16)
        for j in range(1, 16):
            tt(v0[:, :, j], v0[:, :, j - 1], w0[:, :, j], OP.add)
        if tap('s0scan', (128, 3200)):
            nc.sync.dma_start(DT['tap_s0scan'][:], s0)
        # level 1
        nc.sync.dma_start(DT['c1buf'].rearrange("(p g) -> p g", g=200), v0[:, :, 15])
        s1 = pool.tile([100, 256], F32)
        nc.sync.dma_start(s1, DT['c1buf'].rearrange("(p n) -> p n", n=256))
        x1 = pool.tile([100, 256], F32)
        nc.vector.tensor_copy(x1, s1)
        v1 = s1.rearrange("p (g s) -> p g s", s=16)
        w1 = x1.rearrange("p (g s) -> p g s", s=16)
        for j in range(1, 16):
            tt(v1[:, :, j], v1[:, :, j - 1], w1[:, :, j], OP.add)
        # level 2 (in SBUF: groups land on one partition each)
        s2b = pool.tile([100, 16], F32)
        nc.vector.tensor_copy(s2b[:, 0:1], v1[:, 0, 15:16])
        for j in range(1, 16):
            tt(s2b[:, j:j + 1], s2b[:, j - 1:j], v1[:, j, 15:16], OP.add)
        # level 3: bring s2b[:,15] (100 vals) to one partition
        nc.sync.dma_start(DT['l3buf'][0:100, None], s2b[:, 15:16])
        s3 = pool.tile([1, 112], F32)
        nc.vector.memset(s3, 0.0)
        nc.sync.dma_start(s3[:, 0:100], DT['l3buf'][None, 0:100])
        x3 = pool.tile([1, 112], F32)
        nc.vector.tensor_copy(x3, s3)
        v3 = s3.rearrange("p (g s) -> p g s", s=16)
        w3 = x3.rearrange("p (g s) -> p g s", s=16)
        for j in range(1, 16):
            tt(v3[:, :, j], v3[:, :, j - 1], w3[:, :, j], OP.add)
        # level 4: exclusive scan of 7 values
        z4 = pool.tile([1, 8], F32)
        nc.vector.memset(z4, 0.0)
        for i in range(1, 8):
            tt(z4[:, i:i + 1], z4[:, i - 1:i], v3[:, i - 1, 15:16], OP.add)
        # t3 = s3 + bcast(z4[0:7])
        tt(v3, v3, z4[:, 0:7, None].to_broadcast((1, 7, 16)), OP.add)
        # off2
        zcol = pool.tile([1, 1], F32)
        nc.vector.memset(zcol, 0.0)
        nc.sync.dma_start(DT['o2buf'][None, 0:1], zcol)
        nc.sync.dma_start(DT['o2buf'][None, 1:101], s3[:, 0:100])
        off2 = pool.tile([100, 1], F32)
        nc.sync.dma_start(off2, DT['o2buf'][0:100, None])
        ts(s2b, s2b, off2[:, 0:1], None, OP.add)
        # off1
        nc.sync.dma_start(DT['o1buf'][None, 0:1], zcol)
        nc.sync.dma_start(DT['o1buf'][1:1601].rearrange("(p g) -> p g", g=16), s2b)
        off1 = pool.tile([100, 16], F32)
        nc.sync.dma_start(off1, DT['o1buf'][0:1600].rearrange("(p g) -> p g", g=16))
        tt(v1, v1, off1[:, :, None].to_broadcast((100, 16, 16)), OP.add)
        # off0
        nc.sync.dma_start(DT['o0buf'][None, 0:1], zcol)
        nc.sync.dma_start(DT['o0buf'][1:25601].rearrange("(p g) -> p g", g=256), s1)
        off0 = pool.tile([128, 200], F32)
        nc.sync.dma_start(off0, DT['o0buf'][0:25600].rearrange("(p g) -> p g", g=200))
        tt(v0, v0, off0[:, :, None].to_broadcast((128, 200, 16)), OP.add)
        rot = s0  # rot_raw now

        if tap('rot_raw', (128, 3200)):
            nc.sync.dma_start(DT['tap_rot_raw'][:], rot)

        # wrap: rotw = rot - round(rot)
        nvals = pool.tile([128, 3200], F32)
        magic_round(pool, nvals, rot, "wrap")
        rotw = x0  # reuse
        tt(rotw, rot, nvals, OP.subtract)
        nc.sync.dma_start(DT['rotw_t'].rearrange("(p n) -> p n", n=3200), rotw)

        # combtooth: x = SR*rotw / (f0up + 1e-3)
        den = pool.tile([128, 3200], F32, tag="den")
        nc.sync.dma_start(den, DT['den_t'].rearrange("(p n) -> p n", n=3200))
        rden = pool.tile([128, 3200], F32, tag="rden")
        nc.vector.reciprocal(rden, den)
        xs = rot  # reuse as x
        ts(xs, rotw, float(SR), None, OP.mult)
        tt(xs, xs, rden, OP.mult)
        # n = round(x); r = x - n; sign = 1 - 8*d^2 with d = x/2-round(x/2)... (parity)
        magic_round(pool, nvals, xs, "sinc")
        rr = den  # reuse
        tt(rr, xs, nvals, OP.subtract)
        half = rden  # reuse
        ts(half, nvals, 0.5, None, OP.mult)
        hr = pool.tile([128, 3200], F32, tag="hr")
        magic_round(pool, hr, half, "par")
        tt(half, half, hr, OP.subtract)       # d
        tt(half, half, half, OP.mult)         # d^2 in {0, .25}
        ts(half, half, -8.0, 1.0, OP.mult, OP.add)   # sign
        sinp = hr  # reuse
        act(sinp, rr, AF.Sin, scale=float(np.pi))
        tt(sinp, sinp, half, OP.mult)         # numer = sign*sin(pi r)
        dpi = half  # reuse
        ts(dpi, xs, float(np.pi), None, OP.mult)
        mz = rr  # reuse
        ts(mz, dpi, 0.0, None, OP.is_equal)
        tt(dpi, dpi, mz, OP.add)
        rden2 = nvals  # reuse
        nc.vector.reciprocal(rden2, dpi)
        ct = xs  # reuse
        tt(ct, sinp, rden2, OP.mult)
        # where(x==0): ct = ct - ct*mz + mz
        tt(sinp, ct, mz, OP.mult)
        tt(ct, ct, sinp, OP.subtract)
        tt(ct, ct, mz, OP.add)
        nc.sync.dma_start(DT['ct_t'].rearrange("(p n) -> p n", n=3200), ct)

    # ======================================================================
    # S1c: phase rows; transpose combtooth to [s, f] in DRAM
    # ======================================================================
    with ExitStack() as ctx:
        pool = ctx.enter_context(tc.tile_pool(name="s1c", bufs=2))
        ppool = ctx.enter_context(tc.tile_pool(name="s1cp", bufs=1, space="PSUM"))
        ph = pool.tile([1, 800], F32)
        rw = DT['rotw_t'][:]
        nc.sync.dma_start(ph, bass.AP(tensor=rw.tensor, offset=rw.offset,
                                      ap=[[0, 1], [512, 800]]))
        ph2 = pool.tile([1, 800], F32)
        ts(ph2, ph, float(2 * np.pi), None, OP.mult)
        op = DT['o_phase'][:]
        nc.sync.dma_start(bass.AP(tensor=op.tensor, offset=op.offset,
                                  ap=[[0, 1], [1, 800]]), ph2)
        ts(ph2, ph, 2.0, None, OP.mult)       # phase/pi for control net
        nc.sync.dma_start(DT['phrow'][None, :], ph2)

        ident = pool.tile([128, 128], F32)
        make_identity(nc, ident)
        ctf = DT['ct_t'].rearrange("(f s) -> f s", s=512)
        for it in range(7):
            rows, r0 = FT[it], FT0[it]
            cf = pool.tile([128, 512], F32, tag="cf", name="cf")
            nc.sync.dma_start(cf[:rows], ctf[r0:r0 + rows])
            for sb in range(4):
                pst = ppool.tile([128, 128], F32, tag="pst", name="pst")
                nc.tensor.transpose(pst[:, :rows], cf[:rows, sb * 128:(sb + 1) * 128],
                                    ident[:rows, :rows])
                sbf = pool.tile([128, 128], F32, tag="sbf", name="sbf")
                nc.vector.tensor_copy(sbf[:, :rows], pst[:, :rows])
                nc.sync.dma_start(DT['ct_sf'][sb * 128:(sb + 1) * 128, r0:r0 + rows],
                                  sbf[:, :rows])

    # ======================================================================
    # S2: control net  ->  tgd_d (tanh gd), src_d (exp hm), npar_d (exp nm)
    # ======================================================================
    _control_net(nc, tc, DT, tap, ts, tt, act)

    # ======================================================================
    # S3/S4/S5: filters
    # ======================================================================
    _filters(nc, tc, DT, tap, ts, tt, act, magic_round)


def _ln_layout_B(nc, tc, ctx, pool, _unused_ppool, ts, tt, act, DT, xs, g_name, b_name, out_tiles):
    """LayerNorm over 256 features (2 partition tiles) in layout B."""
    from contextlib import ExitStack
    lctx = ExitStack()
    ppool = lctx.enter_context(tc.tile_pool(name="ln_ps_pool", bufs=1, space="PSUM"))
    ones_col = pool.tile([128, 1], F32, tag="ln_ones_col", name="ln_ones_col")
    nc.sync.dma_start(ones_col, DT['ones_col'][:])
    ones_row = pool.tile([1, 128], F32, tag="ln_ones_row", name="ln_ones_row")
    nc.sync.dma_start(ones_row, DT['ones_row'][:])
    gcol = [pool.tile([128, 1], F32, tag=f"ln_g{i}", name=f"ln_g{i}") for i in range(2)]
    bcol = [pool.tile([128, 1], F32, tag=f"ln_b{i}", name=f"ln_b{i}") for i in range(2)]
    for i in range(2):
        nc.sync.dma_start(gcol[i], DT[g_name][i * 128:(i + 1) * 128, None])
        nc.sync.dma_start(bcol[i], DT[b_name][i * 128:(i + 1) * 128, None])
    sq = [pool.tile([128, 800], F32, tag=f"ln_sq{i}", name=f"ln_sq{i}") for i in range(2)]
    for i in range(2):
        act(sq[i], xs[i], AF.Square)
    eps = pool.tile([1, 1], F32, tag="ln_eps", name="ln_eps")
    nc.vector.memset(eps, 1e-5)
    rows = pool.tile([1, 800], F32, tag="ln_mrow", name="ln_mrow")
    vrow = pool.tile([1, 800], F32, tag="ln_vrow", name="ln_vrow")
    for ci, c0 in enumerate(CHUNKS):
        pm = ppool.tile([1, NCHUNK], F32, tag="ln_pm", name="ln_pm")
        pv = ppool.tile([1, NCHUNK], F32, tag="ln_pv", name="ln_pv")
        for i in range(2):
            nc.tensor.matmul(pm, ones_col, xs[i][:, c0:c0 + NCHUNK],
                             start=(i == 0), stop=(i == 1))
            nc.tensor.matmul(pv, ones_col, sq[i][:, c0:c0 + NCHUNK],
                             start=(i == 0), stop=(i == 1))
        ts(rows[:, c0:c0 + NCHUNK], pm, float(1.0 / 256.0), None, OP.mult)
        ts(vrow[:, c0:c0 + NCHUNK], pv, float(1.0 / 256.0), None, OP.mult)
    m2 = pool.tile([1, 800], F32, tag="ln_m2", name="ln_m2")
    tt(m2, rows, rows, OP.mult)
    tt(vrow, vrow, m2, OP.subtract)
    sdr = pool.tile([1, 800], F32, tag="ln_sdr", name="ln_sdr")
    act(sdr, vrow, AF.Sqrt, bias=eps[:, 0:1])
    rstd = m2  # reuse
    nc.vector.reciprocal(rstd, sdr)
    nrow = vrow  # reuse: -mean*rstd
    tt(nrow, rows, rstd, OP.mult)
    ts(nrow, nrow, -1.0, None, OP.mult)
    for ci, c0 in enumerate(CHUNKS):
        pa = ppool.tile([128, NCHUNK], F32, tag="ln_pa", name="ln_pa")
        pb = ppool.tile([128, NCHUNK], F32, tag="ln_pb", name="ln_pb")
        nc.tensor.matmul(pa, ones_row, rstd[:, c0:c0 + NCHUNK], start=True, stop=True)
        nc.tensor.matmul(pb, ones_row, nrow[:, c0:c0 + NCHUNK], start=True, stop=True)
        for i in range(2):
            tmp = pool.tile([128, NCHUNK], F32, tag="ln_tmp", name="ln_tmp")
            tt(tmp, xs[i][:, c0:c0 + NCHUNK], pa, OP.mult)
            tt(tmp, tmp, pb, OP.add)
            act(out_tiles[i][:, c0:c0 + NCHUNK], tmp, AF.Identity,
                bias=bcol[i][:, 0:1], scale=gcol[i][:, 0:1])
    lctx.close()


def _control_net(nc, tc, DT, tap, ts, tt, act):
    from contextlib import ExitStack
    with ExitStack() as ctx:
        pw = ctx.enter_context(tc.tile_pool(name="cn_w", bufs=1))
        pool = ctx.enter_context(tc.tile_pool(name="cn", bufs=1))
        px = ctx.enter_context(tc.tile_pool(name="cn_x", bufs=1))
        from contextlib import ExitStack as _ES
        pctx = _ES()
        ppool = pctx.enter_context(tc.tile_pool(name="cn_p1", bufs=1, space="PSUM"))

        # ---- conv1 ----
        xpad = [pool.tile([128, 802], F32, tag=f"xpad{i}", name=f"xpad{i}") for i in range(2)]
        for i in range(2):
            nc.gpsimd.memset(xpad[i], 0.0)
            nc.sync.dma_start(xpad[i][:, 1:801], DT['units_T'][i * 128:(i + 1) * 128, :])
        w1 = pw.tile([128, 3, 2, 256], F32)     # [ki_part, k, ki_tile, O]
        nc.sync.dma_start(w1, DT['conv1_k'].rearrange("k (kt p) o -> p k kt o", p=128))
        b1c = [pw.tile([128, 1], F32, name=f"b1c{i}") for i in range(2)]
        for i in range(2):
            nc.sync.dma_start(b1c[i], DT['conv1_b'][i * 128:(i + 1) * 128, None])
        y1 = [pool.tile([128, 800], F32, tag=f"y1_{i}", name=f"y1_{i}") for i in range(2)]
        for ot in range(2):
            for c0 in CHUNKS:
                ps = ppool.tile([128, NCHUNK], F32, tag="cv_ps", name="cv_ps")
                first = True
                for k in range(3):
                    for kt in range(2):
                        nc.tensor.matmul(ps, w1[:, k, kt, ot * 128:(ot + 1) * 128],
                                         xpad[kt][:, c0 + k:c0 + k + NCHUNK],
                                         start=first, stop=(k == 2 and kt == 1))
                        first = False
                act(y1[ot][:, c0:c0 + NCHUNK], ps, AF.Identity, bias=b1c[ot][:, 0:1])

        # ---- group norm (4 groups of 64) + leaky relu ----
        gnf = pw.tile([128, 2, 2], F32)
        nc.sync.dma_start(gnf, DT['gn_fwd'].rearrange("(t p) g -> p t g", p=128))
        rs = pool.tile([128, 2, 2], F32)        # [p, tile, (sum, sumsq)]
        sqs = pool.tile([128, 800], F32, tag="gn_sq", name="gn_sq")
        for i in range(2):
            nc.vector.reduce_sum(rs[:, i, 0:1], y1[i], axis=mybir.AxisListType.X)
            act(sqs, y1[i], AF.Square, accum_out=rs[:, i, 1:2])
        pstat = ppool.tile([2, 2, 2], F32, tag="gn_ps", name="gn_ps")
        for i in range(2):
            nc.tensor.matmul(pstat[:, i], gnf[:, i], rs[:, i], start=True, stop=True)
        stats = pool.tile([2, 2, 2], F32)       # [grp_in_tile, tile, (s, sq)]
        nc.vector.tensor_copy(stats, pstat)
        mean = pool.tile([2, 2, 2], F32, name="gn_mean")  # [g, t, (m, rstd)]
        ts(mean[:, :, 0:1].rearrange("a b c -> a (b c)"),
           stats[:, :, 0:1].rearrange("a b c -> a (b c)"), float(1.0 / 51200.0), None, OP.mult)
        ex2 = pool.tile([2, 2], F32, name="gn_ex2")
        ts(ex2, stats[:, :, 1], float(1.0 / 51200.0), None, OP.mult)
        msq = pool.tile([2, 2], F32, name="gn_msq")
        tt(msq, mean[:, :, 0], mean[:, :, 0], OP.mult)
        tt(ex2, ex2, msq, OP.subtract)          # var
        epsg = pool.tile([2, 1], F32, name="gn_eps")
        nc.vector.memset(epsg, 1e-5)
        sd = pool.tile([2, 2], F32, name="gn_sd")
        act(sd, ex2, AF.Sqrt, bias=epsg[:, 0:1])
        nc.vector.reciprocal(mean[:, :, 1], sd)
        gnbc = pw.tile([2, 128], F32)
        nc.sync.dma_start(gnbc, DT['gn_bc'][:])
        pmr = ppool.tile([128, 2, 2], F32, tag="gn_pmr", name="gn_pmr")
        for i in range(2):
            nc.tensor.matmul(pmr[:, i], gnbc, mean[:, i], start=True, stop=True)
        gg = [pw.tile([128, 1], F32, name=f"gn_g{i}") for i in range(2)]
        gb = [pw.tile([128, 1], F32, name=f"gn_b{i}") for i in range(2)]
        for i in range(2):
            nc.sync.dma_start(gg[i], DT['gn_g'][i * 128:(i + 1) * 128, None])
            nc.sync.dma_start(gb[i], DT['gn_b'][i * 128:(i + 1) * 128, None])
        scol = pool.tile([128, 2], F32, name="gn_scol")
        bcolv = pool.tile([128, 2], F32, name="gn_bcol")
        for i in range(2):
            tt(scol[:, i:i + 1], pmr[:, i, 1:2], gg[i], OP.mult)
            tt(bcolv[:, i:i + 1], pmr[:, i, 0:1], scol[:, i:i + 1], OP.mult)
        ts(bcolv, bcolv, -1.0, None, OP.mult)
        for i in range(2):
            tt(bcolv[:, i:i + 1], bcolv[:, i:i + 1], gb[i], OP.add)
        for i in range(2):
            act(y1[i], y1[i], AF.Identity, bias=bcolv[:, i:i + 1], scale=scol[:, i:i + 1])
            tmp = pool.tile([128, 800], F32, tag="lr_tmp", name="lr_tmp")
            ts(tmp, y1[i], 0.01, None, OP.mult)
            tt(y1[i], y1[i], tmp, OP.max)
        if tap('y1', (256, 800)):
            for i in range(2):
                nc.sync.dma_start(DT['tap_y1'][i * 128:(i + 1) * 128], y1[i])

        # ---- conv2 + conditioning ----
        for i in range(2):
            nc.gpsimd.memset(xpad[i][:, 0:1], 0.0)
            nc.gpsimd.memset(xpad[i][:, 801:802], 0.0)
            nc.vector.tensor_copy(xpad[i][:, 1:801], y1[i])
        w2 = pw.tile([128, 3, 2, 256], F32, name="w2t")
        nc.sync.dma_start(w2, DT['conv2_k'].rearrange("k (kt p) o -> p k kt o", p=128))
        b2c = [pw.tile([128, 1], F32, name=f"b2c{i}") for i in range(2)]
        for i in range(2):
            nc.sync.dma_start(b2c[i], DT['bias2'][i * 128:(i + 1) * 128, None])
        # conditioner rows
        lfrow = pool.tile([1, 800], F32, name="lfrow")
        f0row = pool.tile([1, 800], F32, name="f0row")
        nc.sync.dma_start(f0row, DT['f0_ext'][None, 0:800])
        act(lfrow, f0row, AF.Ln, bias=1.0, scale=float(1.0 / 700.0))
        phr = pool.tile([1, 800], F32, name="phr")
        nc.sync.dma_start(phr, DT['phrow'][None, :])
        volr = pool.tile([1, 800], F32, name="volr")
        nc.sync.dma_start(volr, DT['vol'][None, :])
        cw = pw.tile([1, 3, 256], F32, name="cw")
        for ci, nm in enumerate(('f0_w', 'phase_w', 'vol_w')):
            nc.sync.dma_start(cw[:, ci], DT[nm][:])
        crows = [lfrow, phr, volr]
        xs = [px.tile([128, 800], F32, name=f"x_{i}") for i in range(2)]
        for ot in range(2):
            for c0 in CHUNKS:
                ps = ppool.tile([128, NCHUNK], F32, tag="cv_ps", name="cv_ps2")
                first = True
                for k in range(3):
                    for kt in range(2):
                        nc.tensor.matmul(ps, w2[:, k, kt, ot * 128:(ot + 1) * 128],
                                         xpad[kt][:, c0 + k:c0 + k + NCHUNK],
                                         start=first, stop=False)
                        first = False
                for ci in range(3):
                    nc.tensor.matmul(ps, cw[:, ci, ot * 128:(ot + 1) * 128],
                                     crows[ci][:, c0:c0 + NCHUNK],
                                     start=False, stop=(ci == 2))
                act(xs[ot][:, c0:c0 + NCHUNK], ps, AF.Identity, bias=b2c[ot][:, 0:1])

        if tap('x0', (256, 800)):
            for i in range(2):
                nc.sync.dma_start(DT['tap_x0'][i * 128:(i + 1) * 128], xs[i])
        pctx.close()

        # ---- transformer layers ----
        for li in range(3):
            _layer(nc, tc, ctx, DT, li, xs, pool, pw, px, None, ts, tt, act, tap)
            if tap(f'xl{li}', (256, 800)):
                for i in range(2):
                    nc.sync.dma_start(DT[f'tap_xl{li}'][i * 128:(i + 1) * 128], xs[i])

        # ---- final LN + output head ----
        la = [px.tile([128, 800], F32, tag=f"lnout{i}", name=f"fln{i}") for i in range(2)]
        _ln_layout_B(nc, tc, ctx, pool, ppool, ts, tt, act, DT, xs, 'ln_g', 'ln_b', la)
        pctx2 = _ES()
        ppool = pctx2.enter_context(tc.tile_pool(name="cn_pout", bufs=1, space="PSUM"))
        wo = pw.tile([128, 2, 1024], F32, name="outw")
        nc.sync.dma_start(wo, DT['out_w'].rearrange("(kt p) o -> p kt o", p=128))
        for mt in range(8):
            bcol = pool.tile([128, 1], F32, tag="ob", name="ob")
            nc.sync.dma_start(bcol, DT['out_b'][mt * 128:(mt + 1) * 128, None])
            for c0 in CHUNKS:
                ps = ppool.tile([128, NCHUNK], F32, tag="out_ps", name="out_ps")
                for kt in range(2):
                    nc.tensor.matmul(ps, wo[:, kt, mt * 128:(mt + 1) * 128],
                                     la[kt][:, c0:c0 + NCHUNK],
                                     start=(kt == 0), stop=(kt == 1))
                ot = pool.tile([128, NCHUNK], F32, tag="out_sb", name="out_sb")
                if mt < 2:
                    act(ot, ps, AF.Tanh, bias=bcol[:, 0:1])
                    nc.sync.dma_start(DT['tgd_d'][mt * 128:(mt + 1) * 128, c0:c0 + NCHUNK], ot)
                elif mt < 6:
                    act(ot, ps, AF.Exp, bias=bcol[:, 0:1])
                    nc.sync.dma_start(DT['src_d'][(mt - 2) * 128:(mt - 1) * 128, c0:c0 + NCHUNK], ot)
                else:
                    act(ot, ps, AF.Exp, bias=bcol[:, 0:1])
                    nc.sync.dma_start(DT['npar_d'][(mt - 6) * 128:(mt - 5) * 128, c0:c0 + NCHUNK], ot)


def _layer(nc, tc, ctx, DT, li, xs, pool, pw, px, _unused, ts, tt, act, tap):
    from contextlib import ExitStack as _ES
    # ---- LN1 ----
    la = [px.tile([128, 800], F32, tag=f"lnout{i}", name=f"la{i}") for i in range(2)]
    _ln_layout_B(nc, tc, ctx, pool, None, ts, tt, act, DT, xs, f'l{li}_ln1_g', f'l{li}_ln1_b', la)
    pctx = _ES()
    ppool = pctx.enter_context(tc.tile_pool(name=f"l{li}_pa", bufs=1, space="PSUM"))

    # ---- QKV (layout B) + V in layout A ----
    wq = pw.tile([128, 2, 768], F32, tag="wqkv", name="wqkv")
    nc.sync.dma_start(wq, DT[f'l{li}_wqkv'].rearrange("(kt p) o -> p kt o", p=128))
    qkv = [px.tile([128, 800], F32, tag=f"qkv{j}", name=f"qkv{j}") for j in range(6)]
    for j in range(6):
        bcol = pool.tile([128, 1], F32, tag="qb", name="qb")
        nc.sync.dma_start(bcol, DT[f'l{li}_bqkv'][j * 128:(j + 1) * 128, None])
        for c0 in CHUNKS:
            ps = ppool.tile([128, NCHUNK], F32, tag="qkv_ps", name="qkv_ps")
            for kt in range(2):
                nc.tensor.matmul(ps, wq[:, kt, j * 128:(j + 1) * 128],
                                 la[kt][:, c0:c0 + NCHUNK],
                                 start=(kt == 0), stop=(kt == 1))
            act(qkv[j][:, c0:c0 + NCHUNK], ps, AF.Identity, bias=bcol[:, 0:1])
    # V in layout A with an appended ones column per head: [k, 8 heads, 33]
    va = [px.tile([128, 8, 33], F32, tag=f"va{ft}", name=f"va{ft}") for ft in range(7)]
    bav = pool.tile([128, 256], F32, tag="bav", name="bav")
    nc.sync.dma_start(bav, rowbc(DT[f'l{li}_bqkv'][512:768]))
    for ft in range(7):
        rows, r0 = FT[ft], FT0[ft]
        ps = ppool.tile([128, 256], F32, tag="qkv_ps", name="va_ps")
        for kt in range(2):
            nc.tensor.matmul(ps[:rows], la[kt][:, r0:r0 + rows],
                             wq[:, kt, 512:768], start=(kt == 0), stop=(kt == 1))
        nc.vector.memset(va[ft], 1.0)
        tt(va[ft][:rows, :, 0:32], ps[:rows].rearrange("p (h d) -> p h d", d=32),
           bav[:rows].rearrange("p (h d) -> p h d", d=32), OP.add)

    # ---- attention (streamed over k-tiles; V carries an extra ones column
    #      per head so AV also accumulates the softmax denominator) ----
    ones32 = pool.tile([1, 32], F32, tag="ones32", name="ones32")
    nc.sync.dma_start(ones32, DT['ones_row'][:, 0:32])
    onrm = [px.tile([128, 800], F32, tag=f"onrm{g}", name=f"onrm{g}") for g in range(2)]
    for g in range(2):
        for ci, c0 in enumerate(CHUNKS):
            pav = [ppool.tile([33, NCHUNK], F32, tag=f"av_ps{h}", name=f"av_ps{h}")
                   for h in range(4)]
            for kt in range(7):
                rows, r0 = FT[kt], FT0[kt]
                for h in range(4):
                    psc = ppool.tile([128, NCHUNK], F32, tag=f"sc_ps{h % 2}",
                                     name=f"sc_ps{h}")
                    nc.tensor.matmul(
                        psc[:rows], qkv[2 + g][h * 32:(h + 1) * 32, r0:r0 + rows],
                        qkv[g][h * 32:(h + 1) * 32, c0:c0 + NCHUNK],
                        start=True, stop=True, tile_position=(h * 32, 0))
                    pt = px.tile([128, NCHUNK], F32, tag=f"pt{h}", name=f"pt{h}")
                    act(pt[:rows], psc[:rows], AF.Exp)
                    nc.tensor.matmul(pav[h], va[kt][:rows, g * 4 + h, :],
                                     pt[:rows], start=(kt == 0), stop=(kt == 6))
            # rows 0..31 of pav[h] = O^T, row 32 = denominator
            pbc = ppool.tile([128, NCHUNK], F32, tag="bc_ps", name="bc_ps")
            for h in range(4):
                dnh = pool.tile([1, NCHUNK], F32, tag="dnh", name="dnh")
                act(dnh, pav[h][32:33], AF.Identity)
                rech = pool.tile([1, NCHUNK], F32, tag="rech", name="rech")
                nc.vector.reciprocal(rech, dnh)
                nc.tensor.matmul(pbc[h * 32:(h + 1) * 32], ones32, rech,
                                 start=True, stop=True, tile_position=(0, h * 32))
            bcs = pool.tile([128, NCHUNK], F32, tag="bcs", name="bcs")
            act(bcs, pbc, AF.Identity)
            for h in range(4):
                tt(onrm[g][h * 32:(h + 1) * 32, c0:c0 + NCHUNK],
                   pav[h][0:32], bcs[h * 32:(h + 1) * 32], OP.mult)
    # ---- wo + residual ----
    pctx.close()
    pctx = _ES()
    ppool = pctx.enter_context(tc.tile_pool(name=f"l{li}_pb", bufs=1, space="PSUM"))
    wo = pw.tile([128, 2, 256], F32, tag="wo", name="wo")
    nc.sync.dma_start(wo, DT[f'l{li}_wo'].rearrange("(kt p) o -> p kt o", p=128))
    for dt_ in range(2):
        bcol = pool.tile([128, 1], F32, tag="bo", name="bo")
        nc.sync.dma_start(bcol, DT[f'l{li}_bo'][dt_ * 128:(dt_ + 1) * 128, None])
        for c0 in CHUNKS:
            ps = ppool.tile([128, NCHUNK], F32, tag="wo_ps", name="wo_ps")
            for g in range(2):
                nc.tensor.matmul(ps, wo[:, g, dt_ * 128:(dt_ + 1) * 128],
                                 onrm[g][:, c0:c0 + NCHUNK],
                                 start=(g == 0), stop=(g == 1))
            attn = pool.tile([128, NCHUNK], F32, tag="attn", name="attn")
            act(attn, ps, AF.Identity, bias=bcol[:, 0:1])
            tt(xs[dt_][:, c0:c0 + NCHUNK], xs[dt_][:, c0:c0 + NCHUNK], attn, OP.add)

    # ---- FF ----
    h2 = [px.tile([128, 800], F32, tag=f"lnout{i}", name=f"h2_{i}") for i in range(2)]
    _ln_layout_B(nc, tc, ctx, pool, ppool, ts, tt, act, DT, xs, f'l{li}_ln2_g', f'l{li}_ln2_b', h2)
    w1t = pw.tile([128, 2, 1024], F32, tag="ffw1", name="ffw1")
    nc.sync.dma_start(w1t, DT[f'l{li}_w1'].rearrange("(kt p) o -> p kt o", p=128))
    a = [px.tile([128, 800], F32, tag=f"ffa{m}", name=f"ffa{m}") for m in range(8)]
    for m in range(8):
        bcol = pool.tile([128, 1], F32, tag="fb1", name="fb1")
        nc.sync.dma_start(bcol, DT[f'l{li}_b1'][m * 128:(m + 1) * 128, None])
        for c0 in CHUNKS:
            ps = ppool.tile([128, NCHUNK], F32, tag="ff_ps", name="ff_ps")
            for kt in range(2):
                nc.tensor.matmul(ps, w1t[:, kt, m * 128:(m + 1) * 128],
                                 h2[kt][:, c0:c0 + NCHUNK],
                                 start=(kt == 0), stop=(kt == 1))
            act(a[m][:, c0:c0 + NCHUNK], ps, AF.Gelu_apprx_tanh, bias=bcol[:, 0:1])
    w2t = pw.tile([128, 8, 256], F32, tag="ffw2", name="ffw2")
    nc.sync.dma_start(w2t, DT[f'l{li}_w2'].rearrange("(kt p) o -> p kt o", p=128))
    for dt_ in range(2):
        bcol = pool.tile([128, 1], F32, tag="fb2", name="fb2")
        nc.sync.dma_start(bcol, DT[f'l{li}_b2'][dt_ * 128:(dt_ + 1) * 128, None])
        for c0 in CHUNKS:
            ps = ppool.tile([128, NCHUNK], F32, tag="ff2_ps", name="ff2_ps")
            for kt in range(8):
                nc.tensor.matmul(ps, w2t[:, kt, dt_ * 128:(dt_ + 1) * 128],
                                 a[kt][:, c0:c0 + NCHUNK],
                                 start=(kt == 0), stop=(kt == 7))
            ffo = pool.tile([128, NCHUNK], F32, tag="ffo", name="ffo")
            act(ffo, ps, AF.Identity, bias=bcol[:, 0:1])
            tt(xs[dt_][:, c0:c0 + NCHUNK], xs[dt_][:, c0:c0 + NCHUNK], ffo, OP.add)
    pctx.close()


def _filters(nc, tc, DT, tap, ts, tt, act, magic_round):
    from contextlib import ExitStack

    def mm_lhsT_dram(pool, dname, ktiles, mslice, tag):
        """Load lhsT [128, ktiles, mlen] from DRAM matrix (K, M)."""
        m0, mlen = mslice
        t = pool.tile([128, ktiles, mlen], F32, tag=tag, name=tag)
        nc.sync.dma_start(
            t, DT[dname].rearrange("(kt p) m -> p kt m", p=128)[:, :, m0:m0 + mlen])
        return t

    # ======================================================================
    # filter 1: allpass on combtooth  -> h1_sf (SBUF, [512, 800])
    # ======================================================================
    with ExitStack() as ctx:
        pool = ctx.enter_context(tc.tile_pool(name="f1", bufs=2))
        ph1 = ctx.enter_context(tc.tile_pool(name="f1h", bufs=1))
        ppool = ctx.enter_context(tc.tile_pool(name="f1p", bufs=1, space="PSUM"))

        # allpass phase + cos/sin
        tgd = [pool.tile([128, 800], F32, tag=f"tgd{i}", name=f"tgd{i}") for i in range(2)]
        for i in range(2):
            nc.sync.dma_start(tgd[i], DT['tgd_d'][i * 128:(i + 1) * 128])
        if tap('tgd', (256, 800)):
            for i in range(2):
                nc.sync.dma_start(DT['tap_tgd'][i * 128:(i + 1) * 128], tgd[i])
        piL = pool.tile([128, 2, 256], F32, tag="piL", name="piL")
        nc.sync.dma_start(piL, DT['piL'].rearrange("(kt p) m -> p kt m", p=128))
        apre = [pool.tile([128, 800], F32, tag=f"apre{i}", name=f"apre{i}") for i in range(2)]
        apim = [pool.tile([128, 800], F32, tag=f"apim{i}", name=f"apim{i}") for i in range(2)]
        for mt in range(2):
            for c0 in CHUNKS:
                ps = ppool.tile([128, NCHUNK], F32, tag="phi_ps", name="phi_ps")
                first = True
                for kt in range(mt + 1):
                    nc.tensor.matmul(ps, piL[:, kt, mt * 128:(mt + 1) * 128],
                                     tgd[kt][:, c0:c0 + NCHUNK],
                                     start=first, stop=(kt == mt))
                    first = False
                u = pool.tile([128, NCHUNK], F32, tag="phi_u", name="phi_u")
                act(u, ps, AF.Identity, scale=float(1.0 / (2 * np.pi)))
                w = pool.tile([128, NCHUNK], F32, tag="phi_w", name="phi_w")
                magic_round(pool, w, u, "apu")
                tt(w, u, w, OP.subtract)
                act(apim[mt][:, c0:c0 + NCHUNK], w, AF.Sin, scale=float(2 * np.pi))
                ts(u, u, 0.25, None, OP.add)
                magic_round(pool, w, u, "apc")
                tt(w, u, w, OP.subtract)
                act(apre[mt][:, c0:c0 + NCHUNK], w, AF.Sin, scale=float(2 * np.pi))

        if tap('apre', (256, 800)):
            for i in range(2):
                nc.sync.dma_start(DT['tap_apre'][i * 128:(i + 1) * 128], apre[i])
        if tap('apim', (256, 800)):
            for i in range(2):
                nc.sync.dma_start(DT['tap_apim'][i * 128:(i + 1) * 128], apim[i])

        # H1 = M1^T applied to (apre, apim); X1 = FR1^T applied to ct frames
        ct = [pool.tile([128, 800], F32, tag=f"ctt{s}", name=f"ctt{s}") for s in range(4)]
        for s in range(4):
            nc.sync.dma_start(ct[s], DT['ct_sf'][s * 128:(s + 1) * 128])
        Yre = [ph1.tile([128, 802], F32, name=f"yre{b}") for b in range(4)]
        Yim = [ph1.tile([128, 802], F32, name=f"yim{b}") for b in range(4)]
        for b in range(4):
            nc.gpsimd.memset(Yre[b][:, 0:1], 0.0)
            nc.gpsimd.memset(Yre[b][:, 801:802], 0.0)
            nc.gpsimd.memset(Yim[b][:, 0:1], 0.0)
            nc.gpsimd.memset(Yim[b][:, 801:802], 0.0)
        for b in range(4):
            l_rr = mm_lhsT_dram(pool, 'M1_rr', 2, (b * 128, 128), "l_rr")
            l_ir = mm_lhsT_dram(pool, 'M1_ir', 2, (b * 128, 128), "l_ir")
            l_ri = mm_lhsT_dram(pool, 'M1_ri', 2, (b * 128, 128), "l_ri")
            l_ii = mm_lhsT_dram(pool, 'M1_ii', 2, (b * 128, 128), "l_ii")
            f_re = mm_lhsT_dram(pool, 'FR1_re', 4, (b * 128, 128), "f_re")
            f_im = mm_lhsT_dram(pool, 'FR1_im', 4, (b * 128, 128), "f_im")
            for c0 in CHUNKS:
                psh_re = ppool.tile([128, NCHUNK], F32, tag="h1re_ps", name="h1re_ps")
                psh_im = ppool.tile([128, NCHUNK], F32, tag="h1im_ps", name="h1im_ps")
                for kt in range(2):
                    nc.tensor.matmul(psh_re, l_rr[:, kt], apre[kt][:, c0:c0 + NCHUNK],
                                     start=(kt == 0), stop=False)
                    nc.tensor.matmul(psh_im, l_ri[:, kt], apre[kt][:, c0:c0 + NCHUNK],
                                     start=(kt == 0), stop=False)
                for kt in range(2):
                    nc.tensor.matmul(psh_re, l_ir[:, kt], apim[kt][:, c0:c0 + NCHUNK],
                                     start=False, stop=(kt == 1))
                    nc.tensor.matmul(psh_im, l_ii[:, kt], apim[kt][:, c0:c0 + NCHUNK],
                                     start=False, stop=(kt == 1))
                psx_re = ppool.tile([128, NCHUNK], F32, tag="x1re_ps", name="x1re_ps")
                psx_im = ppool.tile([128, NCHUNK], F32, tag="x1im_ps", name="x1im_ps")
                for kt in range(4):
                    nc.tensor.matmul(psx_re, f_re[:, kt], ct[kt][:, c0:c0 + NCHUNK],
                                     start=(kt == 0), stop=(kt == 3))
                    nc.tensor.matmul(psx_im, f_im[:, kt], ct[kt][:, c0:c0 + NCHUNK],
                                     start=(kt == 0), stop=(kt == 3))
                # complex product -> Y (into padded cols 1..801)
                hre = pool.tile([128, NCHUNK], F32, tag="hre", name="hre")
                him = pool.tile([128, NCHUNK], F32, tag="him", name="him")
                nc.vector.tensor_copy(hre, psh_re)
                nc.vector.tensor_copy(him, psh_im)
                t1 = pool.tile([128, NCHUNK], F32, tag="cx1", name="cx1")
                t2 = pool.tile([128, NCHUNK], F32, tag="cx2", name="cx2")
                tt(t1, psx_re, hre, OP.mult)
                tt(t2, psx_im, him, OP.mult)
                tt(Yre[b][:, 1 + c0:1 + c0 + NCHUNK], t1, t2, OP.subtract)
                tt(t1, psx_re, him, OP.mult)
                tt(t2, psx_im, hre, OP.mult)
                tt(Yim[b][:, 1 + c0:1 + c0 + NCHUNK], t1, t2, OP.add)

        h1 = _ola(nc, tc, ctx, DT, pool, ph1, ppool, mm_lhsT_dram, 'O1', 4, Yre, Yim, tap, 'h1')
        for s in range(4):
            nc.sync.dma_start(DT['h1_d'][s * 128:(s + 1) * 128], h1[s])

    # ======================================================================
    # filter 2: variable-window src filter on h1  -> harm (SBUF + DRAM)
    # ======================================================================
    with ExitStack() as ctx:
        pool = ctx.enter_context(tc.tile_pool(name="f2", bufs=1))
        ph2 = ctx.enter_context(tc.tile_pool(name="f2h", bufs=1))
        ppool = ctx.enter_context(tc.tile_pool(name="f2p", bufs=1, space="PSUM"))

        src = [pool.tile([128, 800], F32, tag=f"src{i}", name=f"src{i}") for i in range(4)]
        for i in range(4):
            nc.sync.dma_start(src[i], DT['src_d'][i * 128:(i + 1) * 128])
        # window scale row: s_f = (f0 + 1e-3) / 66150
        f0row = pool.tile([1, 800], F32, name="f2_f0row")
        nc.sync.dma_start(f0row, DT['f0_ext'][None, 0:800])
        srow = pool.tile([1, 800], F32, name="f2_srow")
        ts(srow, f0row, 0.001, float(1.0 / 66150.0), OP.add, OP.mult)
        onesr = pool.tile([1, 128], F32, name="f2_ones")
        nc.sync.dma_start(onesr, DT['ones_row'][:])
        sbc = pool.tile([128, 800], F32, name="f2_sbc")
        for c0 in CHUNKS:
            pb = ppool.tile([128, NCHUNK], F32, tag="f2_sbc_ps", name="f2_sbc_ps")
            nc.tensor.matmul(pb, onesr, srow[:, c0:c0 + NCHUNK], start=True, stop=True)
            nc.vector.tensor_copy(sbc[:, c0:c0 + NCHUNK], pb)
        tcolb = pool.tile([128, 8], F32, name="f2_tcol")
        nc.sync.dma_start(tcolb, DT['tcol'].rearrange("(o p) -> p o", p=128))

        # ir2 = P2^T src ; windowed -> ir2w tiles [128, 800] x 8
        ir2w = [ph2.tile([128, 800], F32, name=f"ir2w{o}") for o in range(8)]
        for o in range(8):
            l_p2 = mm_lhsT_dram(pool, 'P2', 4, (o * 128, 128), "l_p2")
            for c0 in CHUNKS:
                ps = ppool.tile([128, NCHUNK], F32, tag="ir2_ps", name="ir2_ps")
                for kt in range(4):
                    nc.tensor.matmul(ps, l_p2[:, kt], src[kt][:, c0:c0 + NCHUNK],
                                     start=(kt == 0), stop=(kt == 3))
                # window: w = .5 + .5 cos(pi * clip(t*s, -1, 1))
                r = pool.tile([128, NCHUNK], F32, tag="f2_r", name="f2_r")
                ts(r, sbc[:, c0:c0 + NCHUNK], tcolb[:, o:o + 1], None, OP.mult)
                ts(r, r, 1.0, -1.0, OP.min, OP.max)
                ts(r, r, 0.5, 0.25, OP.mult, OP.add)
                rw = pool.tile([128, NCHUNK], F32, tag="f2_rw", name="f2_rw")
                magic_round(pool, rw, r, "f2w")
                tt(rw, r, rw, OP.subtract)
                cw = pool.tile([128, NCHUNK], F32, tag="f2_cw", name="f2_cw")
                act(cw, rw, AF.Sin, scale=float(2 * np.pi))
                ts(cw, cw, 0.5, 0.5, OP.mult, OP.add)
                tt(cw, cw, ps, OP.mult)
                nc.vector.tensor_copy(ir2w[o][:, c0:c0 + NCHUNK], cw)

        if tap('ir2w', (1024, 800)):
            for o in range(8):
                nc.sync.dma_start(DT['tap_ir2w'][o * 128:(o + 1) * 128], ir2w[o])

        # H2 = F2^T ir2w ; X2 = FR2^T h1 ; Y2 = X2*H2
        h1t = [pool.tile([128, 800], F32, tag=f"h1t{s}", name=f"h1t{s}") for s in range(4)]
        for s in range(4):
            nc.sync.dma_start(h1t[s], DT['h1_d'][s * 128:(s + 1) * 128])
        Yre = [ph2.tile([128, 802], F32, name=f"y2re{b}") for b in range(6)]
        Yim = [ph2.tile([128, 802], F32, name=f"y2im{b}") for b in range(6)]
        for b in range(6):
            nc.gpsimd.memset(Yre[b][:, 0:1], 0.0)
            nc.gpsimd.memset(Yre[b][:, 801:802], 0.0)
            nc.gpsimd.memset(Yim[b][:, 0:1], 0.0)
            nc.gpsimd.memset(Yim[b][:, 801:802], 0.0)
        for b in range(6):
            f2re = mm_lhsT_dram(pool, 'F2_re', 8, (b * 128, 128), "f2re")
            f2im = mm_lhsT_dram(pool, 'F2_im', 8, (b * 128, 128), "f2im")
            fr2re = mm_lhsT_dram(pool, 'FR2_re', 4, (b * 128, 128), "fr2re")
            fr2im = mm_lhsT_dram(pool, 'FR2_im', 4, (b * 128, 128), "fr2im")
            for c0 in CHUNKS:
                psh_re = ppool.tile([128, NCHUNK], F32, tag="h2re_ps", name="h2re_ps")
                psh_im = ppool.tile([128, NCHUNK], F32, tag="h2im_ps", name="h2im_ps")
                for kt in range(8):
                    nc.tensor.matmul(psh_re, f2re[:, kt], ir2w[kt][:, c0:c0 + NCHUNK],
                                     start=(kt == 0), stop=(kt == 7))
                    nc.tensor.matmul(psh_im, f2im[:, kt], ir2w[kt][:, c0:c0 + NCHUNK],
                                     start=(kt == 0), stop=(kt == 7))
                psx_re = ppool.tile([128, NCHUNK], F32, tag="x2re_ps", name="x2re_ps")
                psx_im = ppool.tile([128, NCHUNK], F32, tag="x2im_ps", name="x2im_ps")
                for kt in range(4):
                    nc.tensor.matmul(psx_re, fr2re[:, kt], h1t[kt][:, c0:c0 + NCHUNK],
                                     start=(kt == 0), stop=(kt == 3))
                    nc.tensor.matmul(psx_im, fr2im[:, kt], h1t[kt][:, c0:c0 + NCHUNK],
                                     start=(kt == 0), stop=(kt == 3))
                hre = pool.tile([128, NCHUNK], F32, tag="h2re", name="h2re")
                him = pool.tile([128, NCHUNK], F32, tag="h2im", name="h2im")
                nc.vector.tensor_copy(hre, psh_re)
                nc.vector.tensor_copy(him, psh_im)
                t1 = pool.tile([128, NCHUNK], F32, tag="cx21", name="cx21")
                t2 = pool.tile([128, NCHUNK], F32, tag="cx22", name="cx22")
                tt(t1, psx_re, hre, OP.mult)
                tt(t2, psx_im, him, OP.mult)
                tt(Yre[b][:, 1 + c0:1 + c0 + NCHUNK], t1, t2, OP.subtract)
                tt(t1, psx_re, him, OP.mult)
                tt(t2, psx_im, hre, OP.mult)
                tt(Yim[b][:, 1 + c0:1 + c0 + NCHUNK], t1, t2, OP.add)

        harm = _ola(nc, tc, ctx, DT, pool, ph2, ppool, mm_lhsT_dram, 'O2', 6, Yre, Yim, tap, 'harm')
        for s in range(4):
            nc.sync.dma_start(DT['o_harm'][s * 128:(s + 1) * 128], harm[s])
            nc.sync.dma_start(DT['harm_d'][s * 128:(s + 1) * 128], harm[s])

    # ======================================================================
    # filter 3: noise filter; signal = harm + noise
    # ======================================================================
    with ExitStack() as ctx:
        pool = ctx.enter_context(tc.tile_pool(name="f3", bufs=2))
        ph3 = ctx.enter_context(tc.tile_pool(name="f3h", bufs=1))
        ppool = ctx.enter_context(tc.tile_pool(name="f3p", bufs=1, space="PSUM"))

        npar = [pool.tile([128, 800], F32, tag=f"np{i}", name=f"np{i}") for i in range(2)]
        for i in range(2):
            nc.sync.dma_start(npar[i], DT['npar_d'][i * 128:(i + 1) * 128])
        nz = [pool.tile([128, 800], F32, tag=f"nz{s}", name=f"nz{s}") for s in range(4)]
        for s in range(4):
            nc.sync.dma_start(nz[s], DT['noise_T'][s * 128:(s + 1) * 128])
            ts(nz[s], nz[s], 2.0, -1.0, OP.mult, OP.add)
        Yre = [ph3.tile([128, 802], F32, name=f"y3re{b}") for b in range(4)]
        Yim = [ph3.tile([128, 802], F32, name=f"y3im{b}") for b in range(4)]
        for b in range(4):
            nc.gpsimd.memset(Yre[b][:, 0:1], 0.0)
            nc.gpsimd.memset(Yre[b][:, 801:802], 0.0)
            nc.gpsimd.memset(Yim[b][:, 0:1], 0.0)
            nc.gpsimd.memset(Yim[b][:, 801:802], 0.0)
        for b in range(4):
            n_re = mm_lhsT_dram(pool, 'N3_re', 2, (b * 128, 128), "n_re")
            n_im = mm_lhsT_dram(pool, 'N3_im', 2, (b * 128, 128), "n_im")
            f_re = mm_lhsT_dram(pool, 'FR1_re', 4, (b * 128, 128), "f_re3")
            f_im = mm_lhsT_dram(pool, 'FR1_im', 4, (b * 128, 128), "f_im3")
            for c0 in CHUNKS:
                psh_re = ppool.tile([128, NCHUNK], F32, tag="h3re_ps", name="h3re_ps")
                psh_im = ppool.tile([128, NCHUNK], F32, tag="h3im_ps", name="h3im_ps")
                for kt in range(2):
                    nc.tensor.matmul(psh_re, n_re[:, kt], npar[kt][:, c0:c0 + NCHUNK],
                                     start=(kt == 0), stop=(kt == 1))
                    nc.tensor.matmul(psh_im, n_im[:, kt], npar[kt][:, c0:c0 + NCHUNK],
                                     start=(kt == 0), stop=(kt == 1))
                psx_re = ppool.tile([128, NCHUNK], F32, tag="x3re_ps", name="x3re_ps")
                psx_im = ppool.tile([128, NCHUNK], F32, tag="x3im_ps", name="x3im_ps")
                for kt in range(4):
                    nc.tensor.matmul(psx_re, f_re[:, kt], nz[kt][:, c0:c0 + NCHUNK],
                                     start=(kt == 0), stop=(kt == 3))
                    nc.tensor.matmul(psx_im, f_im[:, kt], nz[kt][:, c0:c0 + NCHUNK],
                                     start=(kt == 0), stop=(kt == 3))
                hre = pool.tile([128, NCHUNK], F32, tag="h3re", name="h3re")
                him = pool.tile([128, NCHUNK], F32, tag="h3im", name="h3im")
                nc.vector.tensor_copy(hre, psh_re)
                nc.vector.tensor_copy(him, psh_im)
                t1 = pool.tile([128, NCHUNK], F32, tag="cx31", name="cx31")
                t2 = pool.tile([128, NCHUNK], F32, tag="cx32", name="cx32")
                tt(t1, psx_re, hre, OP.mult)
                tt(t2, psx_im, him, OP.mult)
                tt(Yre[b][:, 1 + c0:1 + c0 + NCHUNK], t1, t2, OP.subtract)
                tt(t1, psx_re, him, OP.mult)
                tt(t2, psx_im, hre, OP.mult)
                tt(Yim[b][:, 1 + c0:1 + c0 + NCHUNK], t1, t2, OP.add)

        nout = _ola(nc, tc, ctx, DT, pool, ph3, ppool, mm_lhsT_dram, 'O1', 4, Yre, Yim, tap, 'noise')
        for s in range(4):
            nc.sync.dma_start(DT['o_noise'][s * 128:(s + 1) * 128], nout[s])
            hh = pool.tile([128, 800], F32, tag="hh", name="hh")
            nc.sync.dma_start(hh, DT['harm_d'][s * 128:(s + 1) * 128])
            tt(hh, hh, nout[s], OP.add)
            nc.sync.dma_start(DT['o_signal'][s * 128:(s + 1) * 128], hh)


def _ola(nc, tc, ctx, DT, pool, outpool, ppool, mm_lhsT_dram, oname, nkt, Yre, Yim, tap, tapname):
    """OLA-folded irfft: out[s, f] = sum_mat sum_kt lhsT^T @ Y[shifted frames]."""
    tt = nc.vector.tensor_tensor
    out = []
    shifts = {'A': 1, 'B': 0, 'C': 2}
    for s in range(4):
        res = outpool.tile([128, 800], F32, tag=f"ola_out{s}", name=f"ola_out{s}")
        lh = {}
        for mk in ('A_re', 'A_im', 'B_re', 'B_im', 'C_re', 'C_im'):
            lh[mk] = mm_lhsT_dram(pool, f'{oname}_{mk}', nkt, (s * 128, 128), f"ola_{mk}")
        for c0 in (0, 400):
            ps = ppool.tile([128, NCHUNK], F32, tag="ola_ps", name="ola_ps")
            first = True
            for mk in ('A_re', 'A_im', 'B_re', 'B_im', 'C_re', 'C_im'):
                sh = shifts[mk[0]]
                Y = Yre if mk.endswith('re') else Yim
                for kt in range(nkt):
                    last = (mk == 'C_im' and kt == nkt - 1)
                    nc.tensor.matmul(ps, lh[mk][:, kt],
                                     Y[kt][:, sh + c0: sh + c0 + NCHUNK],
                                     start=first, stop=last)
                    first = False
            nc.vector.tensor_copy(res[:, c0:c0 + NCHUNK], ps)
        out.append(res)
    if tap(tapname, (512, 800)):
        for s in range(4):
            nc.sync.dma_start(DT[f'tap_{tapname}'][s * 128:(s + 1) * 128], out[s])
    return out


# --------------------------------------------------------------------------
_PROGRAM_CACHE = {}


def _get_program(taps=()):
    key = tuple(sorted(taps))
    if key not in _PROGRAM_CACHE:
        _PROGRAM_CACHE[key] = build_program(taps)
    return _PROGRAM_CACHE[key]


def make_in_maps(inputs, taps=()):
    C = build_consts()
    W = prep_weights(inputs['params'])
    units = np.asarray(inputs['units_frames'], np.float32)
    f0f = np.asarray(inputs['f0_frames'], np.float32)
    vol = np.asarray(inputs['volume_frames'], np.float32)
    noise = np.asarray(inputs['noise_unit'], np.float32)
    B = units.shape[0]
    shared = {}
    for k, v in C.items():
        shared[k] = v
    for k, v in W.items():
        shared[k] = v
    in_maps = []
    for b in range(B):
        m = dict(shared)
        m['units_T'] = np.ascontiguousarray(units[b].T)
        f0e = np.concatenate([f0f[b, :, 0], f0f[b, -1:, 0]])
        m['f0_ext'] = np.ascontiguousarray(f0e)
        wj = (np.arange(BLOCK, dtype=np.float64) / BLOCK).astype(np.float32)
        Ar = (np.float32(1.0) - wj)
        f0u = (f0e[:F, None] * Ar[None, :] + f0e[1:F + 1, None] * wj[None, :]).astype(np.float32)
        m['step_in'] = np.ascontiguousarray((f0u / np.float32(SR)).astype(np.float32).reshape(-1))
        m['vol'] = np.ascontiguousarray(vol[b])
        m['noise_T'] = np.ascontiguousarray(noise[b].reshape(800, 512).T)
        in_maps.append(m)
    return in_maps


def kernel(units_frames, f0_frames, volume_frames, noise_unit, spk_id, params,
           _taps=(), _trace=False):
    inputs = dict(units_frames=units_frames, f0_frames=f0_frames,
                  volume_frames=volume_frames, noise_unit=noise_unit,
                  spk_id=spk_id, params=params)
    nc = _get_program(_taps)
    in_maps = make_in_maps(inputs, _taps)
    B = len(in_maps)
    r = run_bass_kernel_spmd(nc, in_maps, list(range(B)), trace=_trace)
    res = r.results
    sig = np.stack([res[b]['o_signal'].T.reshape(-1) for b in range(B)])
    ph = np.stack([res[b]['o_phase'] for b in range(B)])[..., None]
    harm = np.stack([res[b]['o_harm'].T.reshape(-1) for b in range(B)])
    nois = np.stack([res[b]['o_noise'].T.reshape(-1) for b in range(B)])
    out = (sig.astype(np.float32), ph.astype(np.float32),
           harm.astype(np.float32), nois.astype(np.float32))
    if _taps or _trace:
        return out, res, r
    return out
